# revision 18
# baseline (speedup 1.0000x reference)
"""Bass/Trainium2 kernel for a fused GRU cell.

  r   = sigmoid(x @ W_ir.T + h @ W_hr.T + b_r)
  z   = sigmoid(x @ W_iz.T + h @ W_hz.T + b_z)
  g   = tanh  (x @ W_ih.T + (r*h) @ W_hh.T + b_h)
  h_t = (1-z)*h + z*g

Sharding: data-parallel over the batch (8192 -> 1024 rows per core on 8
NeuronCores), weights replicated, no collectives.

Layout: transposed on-device ([hidden, batch] with hidden on SBUF
partitions) so biases are per-partition scalars and weight tiles land as
natural [K,M] stationary operands. All matmul operands are bf16 (same
1 cycle/row PE rate as float32r but half the DMA/LDWEIGHTS bytes);
accumulation is fp32 PSUM, activations and the final combine are fp32.

Startup: one DMA stream on the sync queue in exact need-order. The
r-gate opens one PSUM group per bank (8 h-tiles, batch-chunk 0); its
x-side runs in a (h-tile, k-tile) wavefront so each matmul only needs
DMA pieces that have already landed, then the h-side runs in k-bands
across all 8 h-tiles so h_prev is consumed as it streams in. Batch-chunk
1 of those h-tiles then runs entirely from SBUF-resident weights (a
~40us DMA catch-up window). The final h-gate group is split into two
half-bank groups so the closing tanh/combine/store chain is short.
"""

import sys

for _p in ("/opt/trn_rl_repo", "/root/.axon_site/_ro/trn_rl_repo"):
    if _p not in sys.path:
        sys.path.append(_p)

import numpy as np
import ml_dtypes

P = 128          # SBUF partitions
BC = 512         # moving free-dim per matmul (one fp32 PSUM bank)
N_CORES = 8

_PROG_CACHE = {}


def build_program(Bc, IN, H):
    """Build the per-core SPMD Bass program (identical on all cores)."""
    from contextlib import ExitStack

    from concourse import bacc, bass, mybir, tile
    from concourse.dt import dt

    KI, KH, NT = IN // P, H // P, H // P
    NJ = KI + KH                 # contraction tiles per gate per h-tile
    QT = NJ                      # one weight slab per (gate, h-tile)
    NB = Bc // BC
    f32, bf16 = dt.float32, dt.bfloat16
    SIG = mybir.ActivationFunctionType.Sigmoid
    TANH = mybir.ActivationFunctionType.Tanh

    nc = bacc.Bacc("TRN2", debug=False)
    xt_d = nc.declare_dram_parameter("xt", [P, KI, Bc], bf16, False)
    hp_d = nc.declare_dram_parameter("hp", [P, KH, Bc], bf16, False)
    wr_d = nc.declare_dram_parameter("wr", [NT, P, QT * P], bf16, False)
    wzh_d = nc.declare_dram_parameter("wzh", [NT, P, 2 * QT * P], bf16, False)
    b_d = nc.declare_dram_parameter("bias", [P, NT * 3], f32, False)
    out_d = nc.declare_dram_parameter("out", [NT, P, Bc], f32, True)

    PRE = min(NT, 8)             # h-tiles resident during the prologue
    KH2 = KH // 2
    HPC = 4                      # h_prev tiles per DMA chunk
    assert KH % HPC == 0 and KH2 % HPC == 0

    with ExitStack() as ctx:
        tc = ctx.enter_context(tile.TileContext(nc))
        res = ctx.enter_context(tc.tile_pool(name="res", bufs=1))
        wp = ctx.enter_context(tc.tile_pool(name="wp", bufs=9))
        pp = ctx.enter_context(
            tc.tile_pool(name="pp", bufs=8, space=bass.MemorySpace.PSUM)
        )
        op = ctx.enter_context(tc.tile_pool(name="op", bufs=3))
        zp = ctx.enter_context(tc.tile_pool(name="zp", bufs=3))

        xt = res.tile([P, KI, Bc], bf16, tag="xt")
        hp = res.tile([P, KH, Bc], bf16, tag="hp")
        rh = res.tile([P, KH, Bc], bf16, tag="rh")
        bias = res.tile([P, NT * 3], f32, tag="bias")

        r_slab = {
            hti: wp.tile([P, QT * P], bf16, tag="w", name=f"rslab{hti}")
            for hti in range(PRE)
        }

        # Prologue DMA stream on sync, in exact need-order (subtile deps
        # let each matmul wait only on the piece that wrote its region):
        # (xt-bc0, r-xside) pairs -> hp c0 -> r-hside 1st half -> hp c1 ->
        # r-hside 2nd half -> hp c2..  -> xt-bc1.
        # Three parallel input streams, each in need-order: x on the scalar
        # queue, h_prev on gpsimd (its output DMAs only start much later),
        # weights on sync — descriptor issue and transfers all overlap.
        nc.scalar.dma_start(out=bias[:], in_=b_d[:])
        for k in range(KI):
            nc.scalar.dma_start(out=xt[:, k, :BC], in_=xt_d[:, k, :BC])
        if NB > 1:
            for k in range(KI):
                nc.scalar.dma_start(out=xt[:, k, BC:], in_=xt_d[:, k, BC:])
        for t in range(0, KH, HPC):
            nc.gpsimd.dma_start(out=hp[:, t : t + HPC, :], in_=hp_d[:, t : t + HPC, :])
        # the very first matmul needs only w(ht0,j0) + xt0-bc0 -> 160KB
        nc.sync.dma_start(out=r_slab[0][:, :P], in_=wr_d[0, :, :P])
        for k in range(PRE):
            lo = P if k == 0 else 0
            nc.sync.dma_start(
                out=r_slab[k][:, lo : KI * P], in_=wr_d[k, :, lo : KI * P]
            )
        for hti in range(PRE):
            nc.sync.dma_start(
                out=r_slab[hti][:, KI * P : (KI + KH2) * P],
                in_=wr_d[hti, :, KI * P : (KI + KH2) * P],
            )
        for hti in range(PRE):
            nc.sync.dma_start(
                out=r_slab[hti][:, (KI + KH2) * P :],
                in_=wr_d[hti, :, (KI + KH2) * P :],
            )

        def slab_dma(w_d, hti):
            s = wp.tile([P, QT * P], bf16, tag="w")
            nc.sync.dma_start(out=s[:], in_=w_d[hti])
            return s

        def mm_run(ps, slab, bc, srch, js, start, stop):
            sl = slice(bc * BC, (bc + 1) * BC)
            last, first = js[-1], js[0]
            for j in js:
                lhs = slab[:, j * P : (j + 1) * P]
                mov = xt[:, j, sl] if j < KI else srch[:, j - KI, sl]
                nc.tensor.matmul(
                    ps[:],
                    lhs,
                    mov,
                    start=(start and j == first),
                    stop=(stop and j == last),
                    skip_group_check=True,
                )

        def r_epilogue(ps, hti, bc):
            sl = slice(bc * BC, (bc + 1) * BC)
            nc.scalar.activation(
                ps[:], ps[:], SIG, bias=bias[:, hti * 3 : hti * 3 + 1]
            )
            nc.vector.tensor_mul(rh[:, hti, sl], ps[:], hp[:, hti, sl])

        # ---- phase R: r = sigmoid(gi_r + gh_r + b_r); rh = r * h ----
        r_ps = {}
        for hti in range(PRE):
            ps = pp.tile([P, BC], f32, tag="ps", name=f"rps{hti}")
            r_ps[hti] = ps
        # stage A: x-side of (ht 0..PRE-1, bc0) as a wavefront over ht+j
        for s in range(PRE + KI - 1):
            for hti in range(max(0, s - KI + 1), min(PRE, s + 1)):
                j = s - hti
                mm_run(r_ps[hti], r_slab[hti], 0, hp, [j],
                       start=(j == 0), stop=False)
        # stage B: h-side in k-bands of HPC across all PRE h-tiles
        for b in range(KH // HPC):
            js = list(range(KI + b * HPC, KI + (b + 1) * HPC))
            closing = b == KH // HPC - 1
            for hti in range(PRE):
                mm_run(r_ps[hti], r_slab[hti], 0, hp, js,
                       start=False, stop=closing)
                if closing:
                    r_epilogue(r_ps[hti], hti, 0)
        # stage C: remaining batch-chunks of the first PRE h-tiles — all
        # weights already in SBUF, no input DMA needed for ~40us.
        for hti in range(PRE):
            for bc in range(1, NB):
                ps = pp.tile([P, BC], f32, tag="ps")
                mm_run(ps, r_slab[hti], bc, hp, list(range(NJ)),
                       start=True, stop=True)
                r_epilogue(ps, hti, bc)
        # stage D: the rest of the h-tiles
        for hti in range(PRE, NT):
            slab = slab_dma(wr_d, hti)
            for bc in range(NB):
                ps = pp.tile([P, BC], f32, tag="ps")
                mm_run(ps, slab, bc, hp, list(range(NJ)), start=True, stop=True)
                r_epilogue(ps, hti, bc)

        # ---- phase ZH: z, g, h_t = h + z*(g - h) ----
        # z+h weights for one h-tile arrive as a single fused DMA: one
        # completion wait per h-tile instead of two.
        for hti in range(NT):
            fused = wp.tile([P, 2 * QT * P], bf16, tag="wzh", bufs=4,
                            name=f"wzh{hti}")
            nc.sync.dma_start(out=fused[:], in_=wzh_d[hti])
            zslab = fused[:, : QT * P]
            hslab = fused[:, QT * P :]
            for bc in range(NB):
                sl = slice(bc * BC, (bc + 1) * BC)
                final = hti == NT - 1 and bc == NB - 1
                psz = pp.tile([P, BC], f32, tag="ps")
                mm_run(psz, zslab, bc, hp, list(range(NJ)), start=True, stop=True)
                zs = zp.tile([P, BC], f32, tag="zs")
                psh = pp.tile([P, BC], f32, tag="ps")
                if not final:
                    mm_run(psh, hslab, bc, rh, list(range(NJ)),
                           start=True, stop=True)
                    # z to SBUF (frees the psz bank; also keeps every DVE op
                    # at a single PSUM operand), tanh in place on PSUM.
                    nc.scalar.activation(
                        zs[:], psz[:], SIG,
                        bias=bias[:, hti * 3 + 1 : hti * 3 + 2],
                    )
                    nc.scalar.activation(
                        psh[:], psh[:], TANH,
                        bias=bias[:, hti * 3 + 2 : hti * 3 + 3],
                    )
                    nc.vector.tensor_sub(psh[:], psh[:], hp[:, hti, sl])
                    nc.vector.tensor_mul(psh[:], zs[:], psh[:])
                    o = op.tile([P, BC], f32, tag="o")
                    nc.vector.tensor_add(o[:], psh[:], hp[:, hti, sl])
                    nc.gpsimd.dma_start(out=out_d[hti, :, sl], in_=o[:])
                else:
                    # Final group: two half-bank accumulation groups so the
                    # tail chain after the very last matmul is halved.
                    nc.scalar.activation(
                        zs[:], psz[:], SIG,
                        bias=bias[:, hti * 3 + 1 : hti * 3 + 2],
                    )
                    # u = (1-z)*h, computed while the h-gate halves are
                    # still accumulating -> only z*g and the final add
                    # trail the very last matmul.
                    u = zp.tile([P, BC], f32, tag="zs")
                    nc.vector.tensor_mul(u[:], zs[:], hp[:, hti, sl])
                    nc.vector.tensor_sub(u[:], hp[:, hti, sl], u[:])
                    H2 = BC // 2
                    psh2 = pp.tile([P, BC], f32, tag="ps")
                    for half, ph in enumerate((psh, psh2)):
                        hsl = slice(bc * BC + half * H2, bc * BC + (half + 1) * H2)
                        psl = slice(0, H2)
                        zsl = slice(half * H2, (half + 1) * H2)
                        for j in range(NJ):
                            lhs = hslab[:, j * P : (j + 1) * P]
                            mov = (xt[:, j, hsl] if j < KI
                                   else rh[:, j - KI, hsl])
                            nc.tensor.matmul(
                                ph[:, psl], lhs, mov,
                                start=(j == 0), stop=(j == NJ - 1),
                                skip_group_check=True,
                            )
                        nc.scalar.activation(
                            ph[:, psl], ph[:, psl], TANH,
                            bias=bias[:, hti * 3 + 2 : hti * 3 + 3],
                        )
                        nc.vector.tensor_mul(ph[:, psl], zs[:, zsl],
                                             ph[:, psl])
                        o = op.tile([P, H2], f32, tag="oe")
                        nc.vector.tensor_add(o[:], ph[:, psl], u[:, zsl])
                        nc.gpsimd.dma_start(out=out_d[hti, :, hsl], in_=o[:])

    nc.compile()
    return nc


def _pack_weight_gate(Wi, Wh):
    """Stack [Wi-tiles; Wh-tiles] -> (NT, P, NJ*P) DMA-slab layout.

    slab[hti][p, j*P + m] = W[hti*P + m, k] with k = (j-th k-tile)*P + p,
    i.e. each 128x128 stationary tile is W.T for that (k-tile, h-tile) block.
    """
    H, IN = Wi.shape
    KI, KH, NT = IN // P, H // P, H // P
    ti = Wi.reshape(NT, P, KI, P).transpose(0, 2, 3, 1)  # (NT, KI, p, m)
    th = Wh.reshape(NT, P, KH, P).transpose(0, 2, 3, 1)  # (NT, KH, p, m)
    cat = np.concatenate([ti, th], axis=1)               # (NT, NJ, p, m)
    NJ = KI + KH
    return np.ascontiguousarray(
        cat.transpose(0, 2, 1, 3).reshape(NT, P, NJ * P)
    ).astype(ml_dtypes.bfloat16)


def _pack_acts(a):
    """(Bc, D) -> (P, D//P, Bc) bf16 with [p, t, b] = a[b, t*P + p]."""
    Bc, D = a.shape
    return np.ascontiguousarray(
        a.T.reshape(D // P, P, Bc).transpose(1, 0, 2)
    ).astype(ml_dtypes.bfloat16)


def run(x_t, h_prev, W_ir, W_iz, W_ih, W_hr, W_hz, W_hh, b_r, b_z, b_h,
        trace=False):
    from concourse.bass_utils import run_bass_kernel_spmd

    x_t = np.asarray(x_t, dtype=np.float32)
    h_prev = np.asarray(h_prev, dtype=np.float32)
    B, IN = x_t.shape
    H = h_prev.shape[1]
    assert B % N_CORES == 0
    Bc = B // N_CORES
    NT = H // P

    key = (Bc, IN, H)
    if key not in _PROG_CACHE:
        _PROG_CACHE[key] = build_program(Bc, IN, H)
    nc = _PROG_CACHE[key]

    wr = _pack_weight_gate(np.asarray(W_ir, np.float32), np.asarray(W_hr, np.float32))
    wz = _pack_weight_gate(np.asarray(W_iz, np.float32), np.asarray(W_hz, np.float32))
    wh = _pack_weight_gate(np.asarray(W_ih, np.float32), np.asarray(W_hh, np.float32))
    wzh = np.ascontiguousarray(np.concatenate([wz, wh], axis=2))
    bias = np.ascontiguousarray(
        np.stack(
            [np.asarray(b_r, np.float32), np.asarray(b_z, np.float32),
             np.asarray(b_h, np.float32)], axis=-1
        ).reshape(NT, P, 3).transpose(1, 0, 2).reshape(P, NT * 3)
    )

    in_maps = []
    for c in range(N_CORES):
        rows = slice(c * Bc, (c + 1) * Bc)
        in_maps.append({
            "xt": _pack_acts(x_t[rows]),
            "hp": _pack_acts(h_prev[rows]),
            "wr": wr, "wzh": wzh, "bias": bias,
        })

    kw = {}
    if trace:
        kw = dict(trace=True, trace_cores=[0])
    res = run_bass_kernel_spmd(nc, in_maps, core_ids=list(range(N_CORES)), **kw)

    outs = []
    for c in range(N_CORES):
        o = res.results[c]["out"]          # (NT, P, Bc)
        outs.append(o.reshape(H, Bc).T)    # (Bc, H)
    full = np.concatenate(outs, axis=0).astype(np.float32)
    return (full, res) if trace else full


def kernel(**inputs):
    return run(**inputs)


# revision 20
# speedup vs baseline: 1.0270x; 1.0270x over previous
"""Bass/Trainium2 kernel for a fused GRU cell.

  r   = sigmoid(x @ W_ir.T + h @ W_hr.T + b_r)
  z   = sigmoid(x @ W_iz.T + h @ W_hz.T + b_z)
  g   = tanh  (x @ W_ih.T + (r*h) @ W_hh.T + b_h)
  h_t = (1-z)*h + z*g

Sharding: data-parallel over the batch (8192 -> 1024 rows per core on 8
NeuronCores), weights replicated, no collectives.

Layout: transposed on-device ([hidden, batch] with hidden on SBUF
partitions) so biases are per-partition scalars and weight tiles land as
natural [K,M] stationary operands. All matmul operands are bf16 (same
1 cycle/row PE rate as float32r but half the DMA/LDWEIGHTS bytes);
accumulation is fp32 PSUM, activations and the final combine are fp32.

Startup: one DMA stream on the sync queue in exact need-order. The
r-gate opens one PSUM group per bank (8 h-tiles, batch-chunk 0); its
x-side runs in a (h-tile, k-tile) wavefront so each matmul only needs
DMA pieces that have already landed, then the h-side runs in k-bands
across all 8 h-tiles so h_prev is consumed as it streams in. Batch-chunk
1 of those h-tiles then runs entirely from SBUF-resident weights (a
~40us DMA catch-up window). The final h-gate group is split into two
half-bank groups so the closing tanh/combine/store chain is short.
"""

import sys

for _p in ("/opt/trn_rl_repo", "/root/.axon_site/_ro/trn_rl_repo"):
    if _p not in sys.path:
        sys.path.append(_p)

import numpy as np
import ml_dtypes

P = 128          # SBUF partitions
BC = 512         # moving free-dim per matmul (one fp32 PSUM bank)
N_CORES = 8

_PROG_CACHE = {}


def build_program(Bc, IN, H):
    """Build the per-core SPMD Bass program (identical on all cores)."""
    from contextlib import ExitStack

    from concourse import bacc, bass, mybir, tile
    from concourse.dt import dt

    KI, KH, NT = IN // P, H // P, H // P
    NJ = KI + KH                 # contraction tiles per gate per h-tile
    QT = NJ                      # one weight slab per (gate, h-tile)
    NB = Bc // BC
    f32, bf16 = dt.float32, dt.bfloat16
    SIG = mybir.ActivationFunctionType.Sigmoid
    TANH = mybir.ActivationFunctionType.Tanh

    nc = bacc.Bacc("TRN2", debug=False)
    xt_d = nc.declare_dram_parameter("xt", [P, KI, Bc], bf16, False)
    hp_d = nc.declare_dram_parameter("hp", [P, KH, Bc], bf16, False)
    wr_d = nc.declare_dram_parameter("wr", [NT, P, QT * P], bf16, False)
    wz_d = nc.declare_dram_parameter("wz", [NT, P, QT * P], bf16, False)
    wh_d = nc.declare_dram_parameter("wh", [NT, P, QT * P], bf16, False)
    b_d = nc.declare_dram_parameter("bias", [P, NT * 3], f32, False)
    out_d = nc.declare_dram_parameter("out", [NT, P, Bc], f32, True)

    PRE = min(NT, 8)             # h-tiles resident during the prologue
    KH2 = KH // 2
    HPC = 4                      # h_prev tiles per DMA chunk
    assert KH % HPC == 0 and KH2 % HPC == 0

    with ExitStack() as ctx:
        tc = ctx.enter_context(tile.TileContext(nc))
        res = ctx.enter_context(tc.tile_pool(name="res", bufs=1))
        wp = ctx.enter_context(tc.tile_pool(name="wp", bufs=12))
        pp = ctx.enter_context(
            tc.tile_pool(name="pp", bufs=8, space=bass.MemorySpace.PSUM)
        )
        op = ctx.enter_context(tc.tile_pool(name="op", bufs=4))
        zp = ctx.enter_context(tc.tile_pool(name="zp", bufs=4))

        xt = res.tile([P, KI, Bc], bf16, tag="xt")
        hp = res.tile([P, KH, Bc], bf16, tag="hp")
        rh = res.tile([P, KH, Bc], bf16, tag="rh")
        bias = res.tile([P, NT * 3], f32, tag="bias")

        r_slab = {
            hti: wp.tile([P, QT * P], bf16, tag="w", name=f"rslab{hti}")
            for hti in range(PRE)
        }

        # Prologue DMA stream on sync, in exact need-order (subtile deps
        # let each matmul wait only on the piece that wrote its region):
        # (xt-bc0, r-xside) pairs -> hp c0 -> r-hside 1st half -> hp c1 ->
        # r-hside 2nd half -> hp c2..  -> xt-bc1.
        nc.scalar.dma_start(out=bias[:], in_=b_d[:])
        for k in range(max(KI, PRE)):
            if k < KI:
                nc.sync.dma_start(out=xt[:, k, :BC], in_=xt_d[:, k, :BC])
            if k < PRE:
                nc.sync.dma_start(
                    out=r_slab[k][:, : KI * P], in_=wr_d[k, :, : KI * P]
                )
        nc.sync.dma_start(out=hp[:, 0:HPC, :], in_=hp_d[:, 0:HPC, :])
        for hti in range(PRE):
            nc.sync.dma_start(
                out=r_slab[hti][:, KI * P : (KI + KH2) * P],
                in_=wr_d[hti, :, KI * P : (KI + KH2) * P],
            )
        nc.sync.dma_start(out=hp[:, HPC : 2 * HPC, :], in_=hp_d[:, HPC : 2 * HPC, :])
        for hti in range(PRE):
            nc.sync.dma_start(
                out=r_slab[hti][:, (KI + KH2) * P :],
                in_=wr_d[hti, :, (KI + KH2) * P :],
            )
        for t in range(2 * HPC, KH, HPC):
            nc.sync.dma_start(out=hp[:, t : t + HPC, :], in_=hp_d[:, t : t + HPC, :])
        if NB > 1:
            for k in range(KI):
                nc.sync.dma_start(out=xt[:, k, BC:], in_=xt_d[:, k, BC:])

        def slab_dma(w_d, hti):
            s = wp.tile([P, QT * P], bf16, tag="w")
            nc.sync.dma_start(out=s[:], in_=w_d[hti])
            return s

        def mm_run(ps, slab, bc, srch, js, start, stop):
            sl = slice(bc * BC, (bc + 1) * BC)
            last, first = js[-1], js[0]
            for j in js:
                lhs = slab[:, j * P : (j + 1) * P]
                mov = xt[:, j, sl] if j < KI else srch[:, j - KI, sl]
                nc.tensor.matmul(
                    ps[:],
                    lhs,
                    mov,
                    start=(start and j == first),
                    stop=(stop and j == last),
                    skip_group_check=True,
                )

        def r_epilogue(ps, hti, bc):
            sl = slice(bc * BC, (bc + 1) * BC)
            nc.scalar.activation(
                ps[:], ps[:], SIG, bias=bias[:, hti * 3 : hti * 3 + 1]
            )
            nc.vector.tensor_mul(rh[:, hti, sl], ps[:], hp[:, hti, sl])

        # ---- phase R: r = sigmoid(gi_r + gh_r + b_r); rh = r * h ----
        r_ps = {}
        for hti in range(PRE):
            ps = pp.tile([P, BC], f32, tag="ps", name=f"rps{hti}")
            r_ps[hti] = ps
        # stage A: x-side of (ht 0..PRE-1, bc0) as a wavefront over ht+j
        for s in range(PRE + KI - 1):
            for hti in range(max(0, s - KI + 1), min(PRE, s + 1)):
                j = s - hti
                mm_run(r_ps[hti], r_slab[hti], 0, hp, [j],
                       start=(j == 0), stop=False)
        # stage B: h-side in k-bands of HPC across all PRE h-tiles
        for b in range(KH // HPC):
            js = list(range(KI + b * HPC, KI + (b + 1) * HPC))
            closing = b == KH // HPC - 1
            for hti in range(PRE):
                mm_run(r_ps[hti], r_slab[hti], 0, hp, js,
                       start=False, stop=closing)
                if closing:
                    r_epilogue(r_ps[hti], hti, 0)
        # stage C: remaining batch-chunks of the first PRE h-tiles — all
        # weights already in SBUF, no input DMA needed for ~40us.
        for hti in range(PRE):
            for bc in range(1, NB):
                ps = pp.tile([P, BC], f32, tag="ps")
                mm_run(ps, r_slab[hti], bc, hp, list(range(NJ)),
                       start=True, stop=True)
                r_epilogue(ps, hti, bc)
        # stage D: the rest of the h-tiles
        for hti in range(PRE, NT):
            slab = slab_dma(wr_d, hti)
            for bc in range(NB):
                ps = pp.tile([P, BC], f32, tag="ps")
                mm_run(ps, slab, bc, hp, list(range(NJ)), start=True, stop=True)
                r_epilogue(ps, hti, bc)

        # ---- phase ZH: z, g, h_t = h + z*(g - h) ----
        for hti in range(NT):
            zslab = slab_dma(wz_d, hti)
            hslab = slab_dma(wh_d, hti)
            for bc in range(NB):
                sl = slice(bc * BC, (bc + 1) * BC)
                final = hti == NT - 1 and bc == NB - 1
                psz = pp.tile([P, BC], f32, tag="ps")
                mm_run(psz, zslab, bc, hp, list(range(NJ)), start=True, stop=True)
                zs = zp.tile([P, BC], f32, tag="zs")
                psh = pp.tile([P, BC], f32, tag="ps")
                if not final:
                    mm_run(psh, hslab, bc, rh, list(range(NJ)),
                           start=True, stop=True)
                    # z to SBUF (frees the psz bank; also keeps every DVE op
                    # at a single PSUM operand), tanh in place on PSUM.
                    nc.scalar.activation(
                        zs[:], psz[:], SIG,
                        bias=bias[:, hti * 3 + 1 : hti * 3 + 2],
                    )
                    nc.scalar.activation(
                        psh[:], psh[:], TANH,
                        bias=bias[:, hti * 3 + 2 : hti * 3 + 3],
                    )
                    nc.vector.tensor_sub(psh[:], psh[:], hp[:, hti, sl])
                    nc.vector.tensor_mul(psh[:], zs[:], psh[:])
                    o = op.tile([P, BC], f32, tag="o")
                    nc.vector.tensor_add(o[:], psh[:], hp[:, hti, sl])
                    nc.gpsimd.dma_start(out=out_d[hti, :, sl], in_=o[:])
                else:
                    # Final group: two half-bank accumulation groups so the
                    # tail chain after the very last matmul is halved.
                    nc.scalar.activation(
                        zs[:], psz[:], SIG,
                        bias=bias[:, hti * 3 + 1 : hti * 3 + 2],
                    )
                    # u = (1-z)*h, computed while the h-gate halves are
                    # still accumulating -> only z*g and the final add
                    # trail the very last matmul.
                    u = zp.tile([P, BC], f32, tag="zs")
                    nc.vector.tensor_mul(u[:], zs[:], hp[:, hti, sl])
                    nc.vector.tensor_sub(u[:], hp[:, hti, sl], u[:])
                    H2 = BC // 2
                    psh2 = pp.tile([P, BC], f32, tag="ps")
                    for half, ph in enumerate((psh, psh2)):
                        hsl = slice(bc * BC + half * H2, bc * BC + (half + 1) * H2)
                        psl = slice(0, H2)
                        zsl = slice(half * H2, (half + 1) * H2)
                        for j in range(NJ):
                            lhs = hslab[:, j * P : (j + 1) * P]
                            mov = (xt[:, j, hsl] if j < KI
                                   else rh[:, j - KI, hsl])
                            nc.tensor.matmul(
                                ph[:, psl], lhs, mov,
                                start=(j == 0), stop=(j == NJ - 1),
                                skip_group_check=True,
                            )
                        nc.scalar.activation(
                            ph[:, psl], ph[:, psl], TANH,
                            bias=bias[:, hti * 3 + 2 : hti * 3 + 3],
                        )
                        nc.vector.tensor_mul(ph[:, psl], zs[:, zsl],
                                             ph[:, psl])
                        o = op.tile([P, H2], f32, tag="oe")
                        nc.vector.tensor_add(o[:], ph[:, psl], u[:, zsl])
                        nc.gpsimd.dma_start(out=out_d[hti, :, hsl], in_=o[:])

    nc.compile()
    return nc


def _pack_weight_gate(Wi, Wh):
    """Stack [Wi-tiles; Wh-tiles] -> (NT, P, NJ*P) DMA-slab layout.

    slab[hti][p, j*P + m] = W[hti*P + m, k] with k = (j-th k-tile)*P + p,
    i.e. each 128x128 stationary tile is W.T for that (k-tile, h-tile) block.
    """
    H, IN = Wi.shape
    KI, KH, NT = IN // P, H // P, H // P
    ti = Wi.reshape(NT, P, KI, P).transpose(0, 2, 3, 1)  # (NT, KI, p, m)
    th = Wh.reshape(NT, P, KH, P).transpose(0, 2, 3, 1)  # (NT, KH, p, m)
    cat = np.concatenate([ti, th], axis=1)               # (NT, NJ, p, m)
    NJ = KI + KH
    return np.ascontiguousarray(
        cat.transpose(0, 2, 1, 3).reshape(NT, P, NJ * P)
    ).astype(ml_dtypes.bfloat16)


def _pack_acts(a):
    """(Bc, D) -> (P, D//P, Bc) bf16 with [p, t, b] = a[b, t*P + p]."""
    Bc, D = a.shape
    return np.ascontiguousarray(
        a.T.reshape(D // P, P, Bc).transpose(1, 0, 2)
    ).astype(ml_dtypes.bfloat16)


def run(x_t, h_prev, W_ir, W_iz, W_ih, W_hr, W_hz, W_hh, b_r, b_z, b_h,
        trace=False):
    from concourse.bass_utils import run_bass_kernel_spmd

    x_t = np.asarray(x_t, dtype=np.float32)
    h_prev = np.asarray(h_prev, dtype=np.float32)
    B, IN = x_t.shape
    H = h_prev.shape[1]
    assert B % N_CORES == 0
    Bc = B // N_CORES
    NT = H // P

    key = (Bc, IN, H)
    if key not in _PROG_CACHE:
        _PROG_CACHE[key] = build_program(Bc, IN, H)
    nc = _PROG_CACHE[key]

    wr = _pack_weight_gate(np.asarray(W_ir, np.float32), np.asarray(W_hr, np.float32))
    wz = _pack_weight_gate(np.asarray(W_iz, np.float32), np.asarray(W_hz, np.float32))
    wh = _pack_weight_gate(np.asarray(W_ih, np.float32), np.asarray(W_hh, np.float32))
    bias = np.ascontiguousarray(
        np.stack(
            [np.asarray(b_r, np.float32), np.asarray(b_z, np.float32),
             np.asarray(b_h, np.float32)], axis=-1
        ).reshape(NT, P, 3).transpose(1, 0, 2).reshape(P, NT * 3)
    )

    in_maps = []
    for c in range(N_CORES):
        rows = slice(c * Bc, (c + 1) * Bc)
        in_maps.append({
            "xt": _pack_acts(x_t[rows]),
            "hp": _pack_acts(h_prev[rows]),
            "wr": wr, "wz": wz, "wh": wh, "bias": bias,
        })

    kw = {}
    if trace:
        kw = dict(trace=True, trace_cores=[0])
    res = run_bass_kernel_spmd(nc, in_maps, core_ids=list(range(N_CORES)), **kw)

    outs = []
    for c in range(N_CORES):
        o = res.results[c]["out"]          # (NT, P, Bc)
        outs.append(o.reshape(H, Bc).T)    # (Bc, H)
    full = np.concatenate(outs, axis=0).astype(np.float32)
    return (full, res) if trace else full


def kernel(**inputs):
    return run(**inputs)
